# revision 51
# baseline (speedup 1.0000x reference)
"""Trainium2 Bass kernel for InputProjection + time/sensor masking + LayerNorm.

Reference computation (B=64, T=4096, C=51, D=64):
    mask[b,t,c] = time_mask[b,t] | sensor_mask[b,c]
    out = LN( einsum('btc,cd->btd', x*(1-mask), W) + einsum('btc,cd->btd', mask, Wm) )

Algebraic restructure (exact):
    With W_b[c,d]   = (1 - sm[b,c]) * W[c,d]
         smWm_b[d]  = sum_c sm[b,c]*Wm[c,d]
         allWm[d]   = sum_c Wm[c,d]
    pre[b,t,d] = sum_c x[b,t,c]*(1-tm[b,t]) * W_b[c,d]
               + 1 * smWm_b[d]
               + tm[b,t] * (allWm - smWm_b)[d]
    (for tm=1 rows the x-term vanishes and pre = allWm exactly, whose LN equals the
     reference's masked-row output; no select needed.)

Device kernel v2 (per core, data-parallel over batch; all I/O bf16):
    - augmented transposed inputs, two batches packed per 128 partitions:
        xaug[pair, half, 53, nj, 128]: rows 0..50 = (x*(1-tm)).T, row 51 = 1,
            row 52 = tm; chunk j holds tokens t = m*nj + j (m = psum partition)
            and is contiguous [53,128] for fast weight load.
        waug[pair, half, 53, D]: rows 0..50 = W_b, 51 = smWm_b, 52 = allWm-smWm_b.
    - per 128-token chunk: one 53-deep bf16 matmul -> PSUM [128t, 64d] fp32
    - per 2 PSUM banks (16 chunks): one wide ACT Copy evict -> SBUF bf16
      (amortizes the ~352-cycle ACT fixed overhead)
    - per chunk pair: one bn_stats whose INPUT AP interleaves the two chunks
      element-wise ([128, 64d, 2]) so the even/odd triple split lands on
      chunk A/B exactly -- full per-chunk stats, no combine chain, and the
      matmul PSUM writes stay contiguous. (The BIR verifier requires 6
      stats/partition per bn_stats, so multi-group 3D bn_stats is out.)
    - per pair: short s/b chain (rsqrt via ACT Sqrt + DVE reciprocal;
      tolerance 2e-2 so no Newton step)
    - per chunk: in-place DVE tensor_scalar (mult,add) apply on the bf16 SBUF
      copy (4x perf mode) then DMA out (each partition writes one contiguous
      nj*64*2B block).
    gamma/beta applied on host only if nontrivial (reference uses 1/0).
"""

import os
import sys
from contextlib import ExitStack

import numpy as np
import ml_dtypes

for _p in ("/opt/trn_rl_repo", "/root/.axon_site/_ro/trn_rl_repo"):
    if os.path.isdir(_p) and _p not in sys.path:
        sys.path.insert(0, _p)

import concourse.bass as bass
import concourse.bacc as bacc
import concourse.mybir as mybir
from concourse import tile
from concourse.bass_utils import run_bass_kernel_spmd

F32 = mybir.dt.float32
BF16 = mybir.dt.bfloat16
AF = mybir.ActivationFunctionType
ALU = mybir.AluOpType

B, T, C, D = 64, 4096, 51, 64
LN_EPS = 1e-5
N_CORES = 8
BPC = B // N_CORES          # batches per core
NPAIR = BPC // 2            # batch pairs per core
CAUG = C + 2                # augmented contraction depth (x rows + ones + tm)
MTILE = 128                 # tokens per matmul chunk (psum partitions)
BANK = 8                    # chunks per PSUM bank (8*64 fp32 = 512 = one bank)
# per-chunk LN-apply engine routing (V=DVE ~203ns, A=ACT ~347ns, G=GPSIMD ?):
# DVE also carries bn_stats (27us), ACT the wide evicts (22us), GPSIMD is idle
APPLY_ROUTE = tuple(os.environ.get("KERNEL_APPLY_ROUTE", "VAGGVAGGVAGVGAGG"))
# split the DVE apply into two single-op tensor_scalars (mult, then add):
# the dual-op (mult,add) uop runs at 1x; single-op tensor_scalar has 2x/4x
# uops which on bf16 SBUF tiles should engage
SPLIT_V_APPLY = os.environ.get("KERNEL_SPLIT_V", "0") == "1"
# packed-token mode: time-masked tokens (30%) have pre == allWm exactly, so
# their LN row is one host-computed constant. Pack only unmasked tokens
# (~2867 of 4096 per batch, +7 sigma < 3072) and scatter back on host.
T_PACK = 3072


def _bn_stats_stream(nc, out_ap, in_ap):
    """bn_stats with a multi-dim input AP treated as ONE positional stream.

    The HW's even/odd triple split is by stream position (dual accumulator
    pipes), so a [128, d, 2] interleaving AP yields chunk-A stats in the even
    triple and chunk-B in the odd one. bass's bn_stats wrapper would treat the
    extra AP dim as a stats "group" and demand a 6*G output (which the BIR
    verifier rejects anyway); emit the raw instruction instead.
    """
    eng = nc.vector
    return eng.add_instruction(
        mybir.InstBNStats(
            name=eng.bass.get_next_instruction_name(),
            ins=[eng.lower_ap(in_ap)],
            outs=[eng.lower_ap(out_ap)],
        )
    )


def build_nc(npair: int, t_len: int, debug: bool = False):
    """Build the per-core Bass program. Identical on all cores (SPMD)."""
    nj = t_len // MTILE                 # chunks per batch
    assert t_len % (MTILE * BANK) == 0, "t_len must be a multiple of 1024"
    nbank = nj // BANK                  # psum banks per batch
    # banks per evict/psum unit: 3-bank units measured 13% slower (too few
    # PSUM units in flight), so cap at 2
    ub = 2 if nbank % 2 == 0 else 1
    nun = nbank // ub                   # units per batch

    nc = bacc.Bacc("TRN2", target_bir_lowering=False, debug=debug)
    # full 128-partition, flat-2D DMA shapes: partial-partition / 3D-AP
    # transfers fall off the distributed DGE path onto a single serialized
    # queue (measured: 930 descriptors on DMA_0 + 5.5us DIRECT2D per
    # dma_start on Sync). The 22 zero partitions cost ~20% extra bytes but
    # keep all 16 DMA engines fed. The per-pair weights ride in cols 0..63
    # of the same buffer (one fewer dma_start on the critical front path).
    xaug_d = nc.dram_tensor("xaug", [npair, 128, D + t_len], BF16,
                            kind="ExternalInput")
    out_d = nc.dram_tensor("out", [2 * npair, t_len, D], BF16,
                           kind="ExternalOutput")

    with tile.TileContext(nc) as tc, ExitStack() as ctx:
        xpool = ctx.enter_context(tc.tile_pool(name="xpool", bufs=4))
        opool = ctx.enter_context(tc.tile_pool(name="opool", bufs=5))
        spool = ctx.enter_context(tc.tile_pool(name="spool", bufs=3))
        tpool = ctx.enter_context(tc.tile_pool(name="tpool", bufs=3))
        psum = ctx.enter_context(
            tc.tile_pool(name="psum", bufs=8 // ub, space="PSUM"))

        for p in range(npair):
            xat = xpool.tile([128, D + t_len], BF16)
            # pair 0: unit-granular input DMA so the first matmuls start
            # after ~256KB instead of the whole ~800KB load. Later pairs
            # load in one piece -- each dma_start costs ~0.65us of
            # serialized DIRECT2D descriptor-gen on the Sync engine, so
            # keep the count low once the pipeline is primed.
            if p == 0:
                for u in range(nbank):
                    cs = slice(0 if u == 0 else D + u * BANK * MTILE,
                               D + (u + 1) * BANK * MTILE)
                    nc.sync.dma_start(xat[:, cs], xaug_d[p, :, cs])
            else:
                nc.sync.dma_start(xat[:], xaug_d[p])
            # cols 0..63 = this pair's waug; chunk j = contiguous 128-col
            # block after that (token t = m*nj + j)
            xa = xat[:, D:].rearrange("k (j m) -> k j m", m=MTILE)

            for i in range(2):
                rb = 64 * i
                # stats triples per chunk pair: slots 0..2 = even chunk,
                # slots 3..5 = odd chunk; count = 64 each.
                stp = spool.tile([128, nj // 2, 6], F32, tag="stp")
                ob = opool.tile([128, nj, D], BF16, tag="ob")
                fin = opool.tile([128, nj, D], BF16, tag="fin")
                for h in range(nun):
                    ps = psum.tile([128, ub, BANK, D], F32, tag="psbank")
                    for hb in range(ub):
                        for q in range(BANK):
                            j = (ub * h + hb) * BANK + q
                            nc.tensor.matmul(
                                ps[:, hb, q, :],
                                xa[rb:rb + CAUG, j, :],
                                xat[rb:rb + CAUG, 0:D],
                                start=True,
                                stop=True,
                            )
                        # interleave-AP bn_stats per chunk pair: stream
                        # A0,B0,A1,B1,... so even/odd triples = chunk A/B
                        for q in range(BANK // 2):
                            g = (ub * h + hb) * (BANK // 2) + q
                            _bn_stats_stream(
                                nc, stp[:, g, :],
                                ps[:, hb, 2 * q:2 * q + 2, :].rearrange(
                                    "p a d -> p d a"))
                    # wide evict: whole unit -> bf16 SBUF in one ACT op
                    nc.scalar.activation(
                        ob[:, ub * h * BANK:ub * (h + 1) * BANK, :], ps[:],
                        AF.Copy)

                # s = rsqrt(var+eps), b = -mu*s per chunk; the even-chunk
                # (slots 1,2) and odd-chunk (slots 4,5) stats are stride-3
                # pairs, so one strided [128, nj/2, 2] view runs the whole
                # chain in 4 ops (no Newton step: tolerance is 2e-2).
                mu = stp[:, :, 1::3]
                M2 = stp[:, :, 2::3]
                veps = tpool.tile([128, nj // 2, 2], F32, tag="veps")
                sq = tpool.tile([128, nj // 2, 2], F32, tag="sq")
                rr = tpool.tile([128, nj // 2, 2], F32, tag="rr")
                bb = tpool.tile([128, nj // 2, 2], F32, tag="bb")
                nc.vector.tensor_scalar(veps[:], M2, 1.0 / D, LN_EPS,
                                        ALU.mult, ALU.add)
                nc.scalar.activation(sq[:], veps[:], AF.Sqrt)
                nc.vector.reciprocal(rr[:], sq[:])
                nc.vector.scalar_tensor_tensor(
                    bb[:], mu, -1.0, rr[:], ALU.mult, ALU.mult)

                for j in range(nj):
                    g = j // 2
                    a = j % 2
                    eng = APPLY_ROUTE[j % len(APPLY_ROUTE)]
                    if eng == "A":
                        nc.scalar.activation(
                            fin[:, j, :], ob[:, j, :], AF.Identity,
                            bias=bb[:, g, a:a + 1], scale=rr[:, g, a:a + 1])
                    else:
                        e = nc.vector if eng == "V" else nc.gpsimd
                        e.tensor_scalar(
                            fin[:, j, :], ob[:, j, :],
                            rr[:, g, a:a + 1], bb[:, g, a:a + 1],
                            ALU.mult, ALU.add)
                odst = out_d[2 * p + i].rearrange("(k j) d -> k j d", k=128)
                # two half-DMAs per batch: the drain starts while the
                # second half's applies still run
                nc.sync.dma_start(odst[:, :nj // 2], fin[:, :nj // 2])
                nc.sync.dma_start(odst[:, nj // 2:], fin[:, nj // 2:])
    nc.compile()
    return nc


def _host_prep(x, W, Wm, time_mask, sensor_mask, n_cores, t_eff, pack_idx):
    """Shard along batch; pack unmasked tokens, transpose/augment (bf16).

    pack_idx: [b, t_eff] int indices of the tokens each batch computes
    (unmasked tokens first, then arbitrary pad indices whose output is
    discarded). With t_eff == t_len this is the identity fallback.
    """
    b, t_len, c = x.shape
    d = W.shape[1]
    npair = (b // n_cores) // 2

    tm = np.ascontiguousarray(time_mask).astype(np.float32)
    sm = np.ascontiguousarray(sensor_mask).astype(np.float32)
    x = np.asarray(x, dtype=np.float32)
    W = np.asarray(W, dtype=np.float32)
    Wm = np.asarray(Wm, dtype=np.float32)

    bi = np.arange(b)[:, None]
    xp = x[bi, pack_idx]                       # [b, t_eff, c]
    tmp_ = tm[bi, pack_idx]                    # [b, t_eff]
    valid = (pack_idx >= 0).astype(np.float32)  # pad rows -> all-zero
    xm = xp * ((1.0 - tmp_) * valid)[:, :, None]
    # pair-packed 128 partitions: batch A rows 0..52, batch B rows 64..116
    xaug = np.zeros((b // 2, 128, t_eff), np.float32)
    xpairs = xm.reshape(b // 2, 2, t_eff, c)
    tmq = (tmp_ * valid).reshape(b // 2, 2, t_eff)
    vq = valid.reshape(b // 2, 2, t_eff)
    for half in range(2):
        rb = 64 * half
        xaug[:, rb:rb + c] = xpairs[:, half].transpose(0, 2, 1)
        xaug[:, rb + c] = vq[:, half]
        xaug[:, rb + c + 1] = tmq[:, half]
    # free layout (j, m): token t = m*nj + j -> chunk j contiguous [*, 128]
    nj = t_eff // MTILE
    xaug = (xaug.reshape(b // 2, 128, MTILE, nj).transpose(0, 1, 3, 2)
            .reshape(b // 2, 128, t_eff))

    allWm = Wm.sum(axis=0)
    smWm = sm @ Wm
    waug_b = np.empty((b, CAUG, d), np.float32)
    waug_b[:, :c] = W[None] * (1.0 - sm)[:, :, None]
    waug_b[:, c] = smWm
    waug_b[:, c + 1] = allWm[None] - smWm
    waug = np.zeros((b // 2, 128, d), np.float32)
    wpairs = waug_b.reshape(b // 2, 2, CAUG, d)
    waug[:, 0:CAUG] = wpairs[:, 0]
    waug[:, 64:64 + CAUG] = wpairs[:, 1]

    # weights ride in cols 0..63 ahead of the token data
    merged = np.concatenate([waug, xaug], axis=2).astype(ml_dtypes.bfloat16)

    in_maps = []
    for m in range(n_cores):
        sl = slice(m * npair, (m + 1) * npair)
        in_maps.append({
            "xaug": np.ascontiguousarray(merged[sl]),
        })
    return in_maps


_NC_CACHE = {}


def kernel(x, W, Wm, gamma, beta, time_mask, sensor_mask):
    x = np.asarray(x)
    b, t_len, c = x.shape
    n_cores = N_CORES
    bpc = b // n_cores
    npair = bpc // 2

    tmb = np.ascontiguousarray(time_mask).astype(bool)
    counts = (~tmb).sum(axis=1)
    t_eff = T_PACK if counts.max() <= T_PACK else t_len

    # pack_idx[b]: indices of unmasked tokens, then -1 pads
    pack_idx = np.full((b, t_eff), -1, np.int64)
    for bb_ in range(b):
        idx = np.flatnonzero(~tmb[bb_])
        if t_eff == t_len:
            pack_idx[bb_] = np.arange(t_len)
        else:
            pack_idx[bb_, :len(idx)] = idx

    key = (npair, t_eff)
    if key not in _NC_CACHE:
        _NC_CACHE[key] = build_nc(npair, t_eff)
    nc = _NC_CACHE[key]

    in_maps = _host_prep(x, W, Wm, time_mask, sensor_mask, n_cores,
                         t_eff, pack_idx)

    trace = bool(int(os.environ.get("KERNEL_TRACE", "0")))
    res = run_bass_kernel_spmd(nc, in_maps, list(range(n_cores)), trace=trace)
    kernel.last_results = res

    dev = np.concatenate(
        [np.asarray(res.results[i]["out"]) for i in range(n_cores)],
        axis=0).astype(np.float32)           # [b, t_eff, d]

    if t_eff == t_len:
        out = dev
    else:
        # masked rows: pre == allWm exactly -> one constant LN row
        allWm = np.asarray(Wm, np.float32).sum(axis=0)
        mu = allWm.mean()
        var = ((allWm - mu) ** 2).mean()
        const_row = (allWm - mu) / np.sqrt(var + LN_EPS)
        out = np.broadcast_to(
            const_row.astype(np.float32), (b, t_len, D)).copy()
        valid = pack_idx >= 0                # [b, t_eff]
        rows = np.repeat(np.arange(b), valid.sum(axis=1))
        out[rows, pack_idx[valid]] = dev[valid]

    gamma = np.asarray(gamma, dtype=np.float32)
    beta = np.asarray(beta, dtype=np.float32)
    if not (np.all(gamma == 1.0) and np.all(beta == 0.0)):
        out = out * gamma + beta
    return out


# revision 52
# speedup vs baseline: 1.0188x; 1.0188x over previous
"""Trainium2 Bass kernel for InputProjection + time/sensor masking + LayerNorm.

Reference computation (B=64, T=4096, C=51, D=64):
    mask[b,t,c] = time_mask[b,t] | sensor_mask[b,c]
    out = LN( einsum('btc,cd->btd', x*(1-mask), W) + einsum('btc,cd->btd', mask, Wm) )

Algebraic restructure (exact):
    With W_b[c,d]   = (1 - sm[b,c]) * W[c,d]
         smWm_b[d]  = sum_c sm[b,c]*Wm[c,d]
         allWm[d]   = sum_c Wm[c,d]
    pre[b,t,d] = sum_c x[b,t,c]*(1-tm[b,t]) * W_b[c,d]
               + 1 * smWm_b[d]
               + tm[b,t] * (allWm - smWm_b)[d]
    (for tm=1 rows the x-term vanishes and pre = allWm exactly, whose LN equals the
     reference's masked-row output; no select needed.)

Device kernel v2 (per core, data-parallel over batch; all I/O bf16):
    - augmented transposed inputs, two batches packed per 128 partitions:
        xaug[pair, half, 53, nj, 128]: rows 0..50 = (x*(1-tm)).T, row 51 = 1,
            row 52 = tm; chunk j holds tokens t = m*nj + j (m = psum partition)
            and is contiguous [53,128] for fast weight load.
        waug[pair, half, 53, D]: rows 0..50 = W_b, 51 = smWm_b, 52 = allWm-smWm_b.
    - per 128-token chunk: one 53-deep bf16 matmul -> PSUM [128t, 64d] fp32
    - per 2 PSUM banks (16 chunks): one wide ACT Copy evict -> SBUF bf16
      (amortizes the ~352-cycle ACT fixed overhead)
    - per chunk pair: one bn_stats whose INPUT AP interleaves the two chunks
      element-wise ([128, 64d, 2]) so the even/odd triple split lands on
      chunk A/B exactly -- full per-chunk stats, no combine chain, and the
      matmul PSUM writes stay contiguous. (The BIR verifier requires 6
      stats/partition per bn_stats, so multi-group 3D bn_stats is out.)
    - per pair: short s/b chain (rsqrt via ACT Sqrt + DVE reciprocal;
      tolerance 2e-2 so no Newton step)
    - per chunk: in-place DVE tensor_scalar (mult,add) apply on the bf16 SBUF
      copy (4x perf mode) then DMA out (each partition writes one contiguous
      nj*64*2B block).
    gamma/beta applied on host only if nontrivial (reference uses 1/0).
"""

import os
import sys
from contextlib import ExitStack

import numpy as np
import ml_dtypes

for _p in ("/opt/trn_rl_repo", "/root/.axon_site/_ro/trn_rl_repo"):
    if os.path.isdir(_p) and _p not in sys.path:
        sys.path.insert(0, _p)

import concourse.bass as bass
import concourse.bacc as bacc
import concourse.mybir as mybir
from concourse import tile
from concourse.bass_utils import run_bass_kernel_spmd

F32 = mybir.dt.float32
BF16 = mybir.dt.bfloat16
AF = mybir.ActivationFunctionType
ALU = mybir.AluOpType

B, T, C, D = 64, 4096, 51, 64
LN_EPS = 1e-5
N_CORES = 8
BPC = B // N_CORES          # batches per core
NPAIR = BPC // 2            # batch pairs per core
CAUG = C + 2                # augmented contraction depth (x rows + ones + tm)
MTILE = 128                 # tokens per matmul chunk (psum partitions)
BANK = 8                    # chunks per PSUM bank (8*64 fp32 = 512 = one bank)
# per-chunk LN-apply engine routing (V=DVE ~203ns, A=ACT ~347ns, G=GPSIMD ?):
# DVE also carries bn_stats (27us), ACT the wide evicts (22us), GPSIMD is idle
APPLY_ROUTE = tuple(os.environ.get("KERNEL_APPLY_ROUTE", "VAGGVAGGVAGVGAGG"))
# split the DVE apply into two single-op tensor_scalars (mult, then add):
# the dual-op (mult,add) uop runs at 1x; single-op tensor_scalar has 2x/4x
# uops which on bf16 SBUF tiles should engage
SPLIT_V_APPLY = os.environ.get("KERNEL_SPLIT_V", "0") == "1"
# packed-token mode: time-masked tokens (30%) have pre == allWm exactly, so
# their LN row is one host-computed constant. Pack only unmasked tokens
# (~2867 of 4096 per batch, +7 sigma < 3072) and scatter back on host.
T_PACK = 3072


def _bn_stats_stream(nc, out_ap, in_ap):
    """bn_stats with a multi-dim input AP treated as ONE positional stream.

    The HW's even/odd triple split is by stream position (dual accumulator
    pipes), so a [128, d, 2] interleaving AP yields chunk-A stats in the even
    triple and chunk-B in the odd one. bass's bn_stats wrapper would treat the
    extra AP dim as a stats "group" and demand a 6*G output (which the BIR
    verifier rejects anyway); emit the raw instruction instead.
    """
    eng = nc.vector
    return eng.add_instruction(
        mybir.InstBNStats(
            name=eng.bass.get_next_instruction_name(),
            ins=[eng.lower_ap(in_ap)],
            outs=[eng.lower_ap(out_ap)],
        )
    )


def build_nc(npair: int, t_len: int, debug: bool = False):
    """Build the per-core Bass program. Identical on all cores (SPMD)."""
    nj = t_len // MTILE                 # chunks per batch
    assert t_len % (MTILE * BANK) == 0, "t_len must be a multiple of 1024"
    nbank = nj // BANK                  # psum banks per batch
    # banks per evict/psum unit: 3-bank units measured 13% slower (too few
    # PSUM units in flight), so cap at 2
    ub = 2 if nbank % 2 == 0 else 1
    nun = nbank // ub                   # units per batch

    nc = bacc.Bacc("TRN2", target_bir_lowering=False, debug=debug)
    # full 128-partition, flat-2D DMA shapes: partial-partition / 3D-AP
    # transfers fall off the distributed DGE path onto a single serialized
    # queue (measured: 930 descriptors on DMA_0 + 5.5us DIRECT2D per
    # dma_start on Sync). The 22 zero partitions cost ~20% extra bytes but
    # keep all 16 DMA engines fed. The per-pair weights ride in cols 0..63
    # of the same buffer (one fewer dma_start on the critical front path).
    xaug_d = nc.dram_tensor("xaug", [npair, 128, D + t_len], BF16,
                            kind="ExternalInput")
    out_d = nc.dram_tensor("out", [2 * npair, t_len, D], BF16,
                           kind="ExternalOutput")

    with tile.TileContext(nc) as tc, ExitStack() as ctx:
        xpool = ctx.enter_context(tc.tile_pool(name="xpool", bufs=3))
        opool = ctx.enter_context(tc.tile_pool(name="opool", bufs=5))
        spool = ctx.enter_context(tc.tile_pool(name="spool", bufs=3))
        tpool = ctx.enter_context(tc.tile_pool(name="tpool", bufs=3))
        psum = ctx.enter_context(
            tc.tile_pool(name="psum", bufs=8 // ub, space="PSUM"))

        for p in range(npair):
            xat = xpool.tile([128, D + t_len], BF16)
            # pair 0: unit-granular input DMA so the first matmuls start
            # after ~256KB instead of the whole ~800KB load. Later pairs
            # load in one piece -- each dma_start costs ~0.65us of
            # serialized DIRECT2D descriptor-gen on the Sync engine, so
            # keep the count low once the pipeline is primed.
            if p == 0:
                for u in range(nbank):
                    cs = slice(0 if u == 0 else D + u * BANK * MTILE,
                               D + (u + 1) * BANK * MTILE)
                    nc.sync.dma_start(xat[:, cs], xaug_d[p, :, cs])
            else:
                nc.sync.dma_start(xat[:], xaug_d[p])
            # cols 0..63 = this pair's waug; chunk j = contiguous 128-col
            # block after that (token t = m*nj + j)
            xa = xat[:, D:].rearrange("k (j m) -> k j m", m=MTILE)

            for i in range(2):
                rb = 64 * i
                # stats triples per chunk pair: slots 0..2 = even chunk,
                # slots 3..5 = odd chunk; count = 64 each.
                stp = spool.tile([128, nj // 2, 6], F32, tag="stp")
                ob = opool.tile([128, nj, D], BF16, tag="ob")
                fin = opool.tile([128, nj, D], BF16, tag="fin")
                for h in range(nun):
                    ps = psum.tile([128, ub, BANK, D], F32, tag="psbank")
                    for hb in range(ub):
                        for q in range(BANK):
                            j = (ub * h + hb) * BANK + q
                            nc.tensor.matmul(
                                ps[:, hb, q, :],
                                xa[rb:rb + CAUG, j, :],
                                xat[rb:rb + CAUG, 0:D],
                                start=True,
                                stop=True,
                            )
                        # interleave-AP bn_stats per chunk pair: stream
                        # A0,B0,A1,B1,... so even/odd triples = chunk A/B
                        for q in range(BANK // 2):
                            g = (ub * h + hb) * (BANK // 2) + q
                            _bn_stats_stream(
                                nc, stp[:, g, :],
                                ps[:, hb, 2 * q:2 * q + 2, :].rearrange(
                                    "p a d -> p d a"))
                    # wide evict: whole unit -> bf16 SBUF in one ACT op
                    nc.scalar.activation(
                        ob[:, ub * h * BANK:ub * (h + 1) * BANK, :], ps[:],
                        AF.Copy)

                # s = rsqrt(var+eps), b = -mu*s per chunk; the even-chunk
                # (slots 1,2) and odd-chunk (slots 4,5) stats are stride-3
                # pairs, so one strided [128, nj/2, 2] view runs the whole
                # chain in 4 ops (no Newton step: tolerance is 2e-2).
                mu = stp[:, :, 1::3]
                M2 = stp[:, :, 2::3]
                veps = tpool.tile([128, nj // 2, 2], F32, tag="veps")
                sq = tpool.tile([128, nj // 2, 2], F32, tag="sq")
                rr = tpool.tile([128, nj // 2, 2], F32, tag="rr")
                bb = tpool.tile([128, nj // 2, 2], F32, tag="bb")
                nc.vector.tensor_scalar(veps[:], M2, 1.0 / D, LN_EPS,
                                        ALU.mult, ALU.add)
                nc.scalar.activation(sq[:], veps[:], AF.Sqrt)
                nc.vector.reciprocal(rr[:], sq[:])
                nc.vector.scalar_tensor_tensor(
                    bb[:], mu, -1.0, rr[:], ALU.mult, ALU.mult)

                for j in range(nj):
                    g = j // 2
                    a = j % 2
                    eng = APPLY_ROUTE[j % len(APPLY_ROUTE)]
                    if eng == "A":
                        nc.scalar.activation(
                            fin[:, j, :], ob[:, j, :], AF.Identity,
                            bias=bb[:, g, a:a + 1], scale=rr[:, g, a:a + 1])
                    else:
                        e = nc.vector if eng == "V" else nc.gpsimd
                        e.tensor_scalar(
                            fin[:, j, :], ob[:, j, :],
                            rr[:, g, a:a + 1], bb[:, g, a:a + 1],
                            ALU.mult, ALU.add)
                odst = out_d[2 * p + i].rearrange("(k j) d -> k j d", k=128)
                # two half-DMAs per batch: the drain starts while the
                # second half's applies still run
                nc.sync.dma_start(odst[:, :nj // 2], fin[:, :nj // 2])
                nc.sync.dma_start(odst[:, nj // 2:], fin[:, nj // 2:])
    nc.compile()
    return nc


def _host_prep(x, W, Wm, time_mask, sensor_mask, n_cores, t_eff, pack_idx):
    """Shard along batch; pack unmasked tokens, transpose/augment (bf16).

    pack_idx: [b, t_eff] int indices of the tokens each batch computes
    (unmasked tokens first, then arbitrary pad indices whose output is
    discarded). With t_eff == t_len this is the identity fallback.
    """
    b, t_len, c = x.shape
    d = W.shape[1]
    npair = (b // n_cores) // 2

    tm = np.ascontiguousarray(time_mask).astype(np.float32)
    sm = np.ascontiguousarray(sensor_mask).astype(np.float32)
    x = np.asarray(x, dtype=np.float32)
    W = np.asarray(W, dtype=np.float32)
    Wm = np.asarray(Wm, dtype=np.float32)

    bi = np.arange(b)[:, None]
    xp = x[bi, pack_idx]                       # [b, t_eff, c]
    tmp_ = tm[bi, pack_idx]                    # [b, t_eff]
    valid = (pack_idx >= 0).astype(np.float32)  # pad rows -> all-zero
    xm = xp * ((1.0 - tmp_) * valid)[:, :, None]
    # pair-packed 128 partitions: batch A rows 0..52, batch B rows 64..116
    xaug = np.zeros((b // 2, 128, t_eff), np.float32)
    xpairs = xm.reshape(b // 2, 2, t_eff, c)
    tmq = (tmp_ * valid).reshape(b // 2, 2, t_eff)
    vq = valid.reshape(b // 2, 2, t_eff)
    for half in range(2):
        rb = 64 * half
        xaug[:, rb:rb + c] = xpairs[:, half].transpose(0, 2, 1)
        xaug[:, rb + c] = vq[:, half]
        xaug[:, rb + c + 1] = tmq[:, half]
    # free layout (j, m): token t = m*nj + j -> chunk j contiguous [*, 128]
    nj = t_eff // MTILE
    xaug = (xaug.reshape(b // 2, 128, MTILE, nj).transpose(0, 1, 3, 2)
            .reshape(b // 2, 128, t_eff))

    allWm = Wm.sum(axis=0)
    smWm = sm @ Wm
    waug_b = np.empty((b, CAUG, d), np.float32)
    waug_b[:, :c] = W[None] * (1.0 - sm)[:, :, None]
    waug_b[:, c] = smWm
    waug_b[:, c + 1] = allWm[None] - smWm
    waug = np.zeros((b // 2, 128, d), np.float32)
    wpairs = waug_b.reshape(b // 2, 2, CAUG, d)
    waug[:, 0:CAUG] = wpairs[:, 0]
    waug[:, 64:64 + CAUG] = wpairs[:, 1]

    # weights ride in cols 0..63 ahead of the token data
    merged = np.concatenate([waug, xaug], axis=2).astype(ml_dtypes.bfloat16)

    in_maps = []
    for m in range(n_cores):
        sl = slice(m * npair, (m + 1) * npair)
        in_maps.append({
            "xaug": np.ascontiguousarray(merged[sl]),
        })
    return in_maps


_NC_CACHE = {}


def kernel(x, W, Wm, gamma, beta, time_mask, sensor_mask):
    x = np.asarray(x)
    b, t_len, c = x.shape
    n_cores = N_CORES
    bpc = b // n_cores
    npair = bpc // 2

    tmb = np.ascontiguousarray(time_mask).astype(bool)
    counts = (~tmb).sum(axis=1)
    t_eff = T_PACK if counts.max() <= T_PACK else t_len

    # pack_idx[b]: indices of unmasked tokens, then -1 pads
    pack_idx = np.full((b, t_eff), -1, np.int64)
    for bb_ in range(b):
        idx = np.flatnonzero(~tmb[bb_])
        if t_eff == t_len:
            pack_idx[bb_] = np.arange(t_len)
        else:
            pack_idx[bb_, :len(idx)] = idx

    key = (npair, t_eff)
    if key not in _NC_CACHE:
        _NC_CACHE[key] = build_nc(npair, t_eff)
    nc = _NC_CACHE[key]

    in_maps = _host_prep(x, W, Wm, time_mask, sensor_mask, n_cores,
                         t_eff, pack_idx)

    trace = bool(int(os.environ.get("KERNEL_TRACE", "0")))
    res = run_bass_kernel_spmd(nc, in_maps, list(range(n_cores)), trace=trace)
    kernel.last_results = res

    dev = np.concatenate(
        [np.asarray(res.results[i]["out"]) for i in range(n_cores)],
        axis=0).astype(np.float32)           # [b, t_eff, d]

    if t_eff == t_len:
        out = dev
    else:
        # masked rows: pre == allWm exactly -> one constant LN row
        allWm = np.asarray(Wm, np.float32).sum(axis=0)
        mu = allWm.mean()
        var = ((allWm - mu) ** 2).mean()
        const_row = (allWm - mu) / np.sqrt(var + LN_EPS)
        out = np.broadcast_to(
            const_row.astype(np.float32), (b, t_len, D)).copy()
        valid = pack_idx >= 0                # [b, t_eff]
        rows = np.repeat(np.arange(b), valid.sum(axis=1))
        out[rows, pack_idx[valid]] = dev[valid]

    gamma = np.asarray(gamma, dtype=np.float32)
    beta = np.asarray(beta, dtype=np.float32)
    if not (np.all(gamma == 1.0) and np.all(beta == 0.0)):
        out = out * gamma + beta
    return out
